# revision 7
# baseline (speedup 1.0000x reference)
"""Multi-task MoE routing (nn_CGC_69836168233304) on 8 TRN2 NeuronCores.

Reference math:
  h[g,e] = relu(x @ W[g,e] + b[g,e])                   12 experts (3 groups x 4)
  sel_t  = softmax(x @ Wg[t] + bg[t])   over 8 cols    t in {0,1}
  sel_s  = softmax(x @ Wgs + bgs)       over 12 cols
  out_t  = sum_m sel_t[:,m] * concat(h[t], h[2])[m]    t in {0,1}
  out_s  = sum_m sel_s[:,m] * concat(h[0],h[1],h[2])[m]

Sharding: data-parallel over batch B=16384 -> 2048 rows/core; every core holds
all 12 expert weights (streamed from HBM, double-buffered, twice per exec) and
produces its batch shard of all three outputs; host concatenates shards (no
collectives).

Kernel structure (per core), sized against HW-measured op rates:
  - PE: 12e x 16bt x (8k x 2 half-bank) fp16 matmuls N=512 into 2-bank PSUM
    tiles ([128,1024] fp32). This is the bound: 3072 streams x ~261 ns (the
    ~213 ns/MM warm roofline degrades to ~261 under the 8-core P0 power
    throttle, HW-measured) ~= 800 us. No bias/gate matmul rides on the PE
    hot path:
      * expert bias enters via ONE DVE tensor_tensor add per (bt,e) against a
        precomputed broadcast bias tile (built once by 24 ones-matmuls + ACT
        copies), not 384 K=1 PE matmuls;
      * gate matmuls (N=28) are interleaved into the e=0 chains, reusing the
        stationary x tile, so they cost only their tiny streams.
  - Epilogue per (bt,e), balanced DVE~530us / ACT~570us, both < PE:
      zb = zp + bias_bcast      (DVE tensor_tensor, PSUM src, 1x, ~1.25us)
      per contribution (28 per bt over 12 experts):
        t/acc = Relu(sel_col * zb)   (ACT, per-partition scale AP; s>0 so
                                      s*relu(z)=relu(s*z); ~1.26us)
        acc += t                      (DVE fp16 tensor_tensor, 2x mode ~0.72us)
    First contribution per accumulator writes it directly; e=11 contributions
    DMA each finished [128,1024] fp16 acc straight to DRAM.
  - fp16 everywhere off-PSUM (measured rel err ~1.2e-3 vs fp32 reference).
"""

import numpy as np

import concourse.bacc as bacc
import concourse.mybir as mybir
import concourse.tile as tile
from concourse.bass_utils import run_bass_kernel_spmd

F32 = mybir.dt.float32
F16 = mybir.dt.float16
AL = mybir.AluOpType
AF = mybir.ActivationFunctionType

N_CORES = 8
B, D, O = 16384, 1024, 1024
BC = B // N_CORES
NE = 12  # experts
NG = 28  # gate columns: 8 (task0) + 8 (task1) + 12 (shared)
SEGS = ((0, 8), (8, 16), (16, 28))
G = 8  # batch tiles per accumulator group
KB = D // 128
N_BT = BC // 128
N_GRP = N_BT // G


def _contribs(e):
    """(out_k, sel_col) pairs for expert e. Gate col order: t0=[g0e0..3,g2e0..3],
    t1=[g1e0..3,g2e0..3], shared=[g0,g1,g2]."""
    if e < 4:
        return [(0, e), (2, 16 + e)]
    if e < 8:
        return [(1, 8 + (e - 4)), (2, 20 + (e - 4))]
    return [(0, 4 + (e - 8)), (1, 12 + (e - 8)), (2, 24 + (e - 8))]


def _build(reps=1):
    dt = F16

    nc = bacc.Bacc("TRN2", target_bir_lowering=False, debug=False)

    xT_d = nc.dram_tensor("xT", [D, BC], dt, kind="ExternalInput")
    w_d = nc.dram_tensor("We", [NE, D, O], dt, kind="ExternalInput")
    b_d = nc.dram_tensor("Be", [1, NE, O], dt, kind="ExternalInput")
    wg_d = nc.dram_tensor("Wgc", [D, NG], dt, kind="ExternalInput")
    bg_d = nc.dram_tensor("bgc", [1, NG], dt, kind="ExternalInput")
    out_d = [
        nc.dram_tensor(f"out{k}", [BC, O], F16, kind="ExternalOutput")
        for k in range(3)
    ]

    with tile.TileContext(nc) as tc:
        with (
            tc.tile_pool(name="big", bufs=1) as big,
            tc.tile_pool(name="wpool", bufs=2) as wpool,
            tc.tile_pool(name="accp", bufs=1) as accp,
            tc.tile_pool(name="tmpp", bufs=4) as tmpp,
            tc.tile_pool(name="gatep", bufs=2) as gatep,
            tc.tile_pool(name="psum", bufs=3, space="PSUM") as psum,
            tc.tile_pool(name="psumg", bufs=2, space="PSUM") as psumg,
        ):
            # --- resident staging ---
            x_sb = big.tile([128, KB, BC], dt)
            nc.sync.dma_start(
                x_sb[:], xT_d.ap().rearrange("(k p) b -> p k b", p=128)
            )
            wg_sb = big.tile([128, KB, NG], dt)
            nc.sync.dma_start(
                wg_sb[:], wg_d.ap().rearrange("(k p) g -> p k g", p=128)
            )
            bg_sb = big.tile([1, NG], dt)
            nc.sync.dma_start(bg_sb[:], bg_d.ap())
            be_sb = big.tile([1, NE, O], dt)
            nc.sync.dma_start(be_sb[:], b_d.ap())
            ones_sb = big.tile([1, 128], dt)
            nc.vector.memset(ones_sb[:], 1.0)

            sel_sb = big.tile([128, N_BT, NG], F32)
            bb_sb = big.tile([128, NE, O], dt)

            for _rep in range(reps):
                _emit_body(
                    nc, x_sb, wg_sb, bg_sb, be_sb, ones_sb, sel_sb, bb_sb,
                    wpool, accp, tmpp, gatep, psum, psumg, w_d, out_d, dt,
                )

    nc.compile()
    return nc


def _emit_body(
    nc, x_sb, wg_sb, bg_sb, be_sb, ones_sb, sel_sb, bb_sb,
    wpool, accp, tmpp, gatep, psum, psumg, w_d, out_d, dt,
):
    # --- broadcast bias tiles: bb[e] = ones(128,1) x b[e] ---
    for e in range(NE):
        for h in range(2):
            pb = psumg.tile([128, 512], F32, tag="aux")
            nc.tensor.matmul(
                pb[:],
                ones_sb[:],
                be_sb[:, e, h * 512 : (h + 1) * 512],
                start=True,
                stop=True,
            )
            nc.scalar.activation(
                bb_sb[:, e, h * 512 : (h + 1) * 512], pb[:], AF.Copy
            )

    # --- experts + gated accumulation; gates ride inside the e=0 chains ---
    for grp in range(N_GRP):
        bt0 = grp * G
        accs = {}
        for e in range(NE):
            w_sb = wpool.tile([128, KB, O], dt, tag="w")
            nc.sync.dma_start(
                w_sb[:],
                w_d.ap()[e].rearrange("(k p) o -> p k o", p=128),
            )
            for bt in range(bt0, bt0 + G):
                zp = psum.tile([128, O], F32)
                pg = None
                if e == 0:
                    pg = psumg.tile([128, 512], F32, tag="aux")
                for k in range(KB):
                    xs = x_sb[:, k, bt * 128 : (bt + 1) * 128]
                    for h in range(2):
                        nc.tensor.matmul(
                            zp[:, h * 512 : (h + 1) * 512],
                            xs,
                            w_sb[:, k, h * 512 : (h + 1) * 512],
                            start=(k == 0),
                            stop=(k == KB - 1),
                        )
                    if e == 0:
                        nc.tensor.matmul(
                            pg[:, :NG],
                            xs,
                            wg_sb[:, k, :],
                            start=(k == 0),
                            stop=False,
                        )
                if e == 0:
                    # gate bias + segment softmax -> sel_sb[:, bt, :]
                    nc.tensor.matmul(
                        pg[:, :NG], ones_sb[:], bg_sb[:],
                        start=False, stop=True,
                    )
                    et = gatep.tile([128, NG], F32)
                    nc.scalar.activation(et[:], pg[:, :NG], AF.Exp)
                    for s0, s1 in SEGS:
                        den = gatep.tile([128, 1], F32, tag="den")
                        nc.vector.tensor_reduce(
                            den[:], et[:, s0:s1], mybir.AxisListType.X, AL.add
                        )
                        rden = gatep.tile([128, 1], F32, tag="rden")
                        nc.vector.reciprocal(rden[:], den[:])
                        nc.vector.tensor_scalar(
                            sel_sb[:, bt, s0:s1], et[:, s0:s1], rden[:],
                            None, AL.mult,
                        )
                zb = tmpp.tile([128, O], dt, tag="zb")
                nc.vector.tensor_tensor(zb[:], zp[:], bb_sb[:, e, :], AL.add)
                for k, col in _contribs(e):
                    sc = sel_sb[:, bt, col : col + 1]
                    key = (k, bt)
                    if key not in accs:
                        a = accp.tile([128, O], dt, tag=f"acc{k}_{bt - bt0}")
                        accs[key] = a
                        nc.scalar.activation(a[:], zb[:], AF.Relu, 0.0, sc)
                    else:
                        a = accs[key]
                        t = tmpp.tile([128, O], dt, tag="t")
                        nc.scalar.activation(t[:], zb[:], AF.Relu, 0.0, sc)
                        nc.vector.tensor_tensor(a[:], a[:], t[:], AL.add)
                    if e == NE - 1:
                        nc.sync.dma_start(
                            out_d[k].ap()[bt * 128 : (bt + 1) * 128, :],
                            a[:],
                        )


_NC_CACHE = None


def make_in_maps(x, W, b, Wg, bg, Wgs, bgs):
    x = np.asarray(x, dtype=np.float32)
    np_dt = np.float16
    shared = {
        "We": np.ascontiguousarray(np.asarray(W).reshape(NE, D, O)).astype(np_dt),
        "Be": np.asarray(b).reshape(1, NE, O).astype(np_dt),
        "Wgc": np.concatenate(
            [np.asarray(Wg)[0], np.asarray(Wg)[1], np.asarray(Wgs)], axis=1
        ).astype(np_dt),
        "bgc": np.concatenate(
            [np.asarray(bg)[0], np.asarray(bg)[1], np.asarray(bgs)]
        )[None, :].astype(np_dt),
    }
    in_maps = []
    for c in range(N_CORES):
        m = dict(shared)
        m["xT"] = np.ascontiguousarray(
            x[c * BC : (c + 1) * BC].T
        ).astype(np_dt)
        in_maps.append(m)
    return in_maps


def _gather(res):
    return tuple(
        np.concatenate(
            [res.results[c][f"out{k}"] for c in range(N_CORES)], axis=0
        ).astype(np.float32)
        for k in range(3)
    )


def kernel(x, W, b, Wg, bg, Wgs, bgs):
    global _NC_CACHE
    if _NC_CACHE is None:
        _NC_CACHE = _build()
    nc = _NC_CACHE

    in_maps = make_in_maps(x, W, b, Wg, bg, Wgs, bgs)
    res = run_bass_kernel_spmd(nc, in_maps, list(range(N_CORES)))
    return _gather(res)


# revision 8
# speedup vs baseline: 1.1635x; 1.1635x over previous
"""Multi-task MoE routing (nn_CGC_69836168233304) on 8 TRN2 NeuronCores.

Reference math:
  h[g,e] = relu(x @ W[g,e] + b[g,e])                   12 experts (3 groups x 4)
  sel_t  = softmax(x @ Wg[t] + bg[t])   over 8 cols    t in {0,1}
  sel_s  = softmax(x @ Wgs + bgs)       over 12 cols
  out_t  = sum_m sel_t[:,m] * concat(h[t], h[2])[m]    t in {0,1}
  out_s  = sum_m sel_s[:,m] * concat(h[0],h[1],h[2])[m]

Sharding: data-parallel over batch B=16384 -> 2048 rows/core; every core holds
all 12 expert weights (streamed from HBM, double-buffered, twice per exec) and
produces its batch shard of all three outputs; host concatenates shards (no
collectives).

Kernel structure (per core), sized against HW-measured op rates:
  - PE: 12e x 16bt x (8k x 2 half-bank) fp16 matmuls N=512 into 2-bank PSUM
    tiles ([128,1024] fp32). This is the bound: 3072 streams x ~261 ns (the
    ~213 ns/MM warm roofline degrades to ~261 under the 8-core P0 power
    throttle, HW-measured) ~= 800 us. No bias/gate matmul rides on the PE
    hot path:
      * expert bias enters via ONE DVE tensor_tensor add per (bt,e) against a
        precomputed broadcast bias tile (built once by 24 ones-matmuls + ACT
        copies), not 384 K=1 PE matmuls;
      * gate matmuls (N=28) are interleaved into the e=0 chains, reusing the
        stationary x tile, so they cost only their tiny streams.
  - Epilogue per (bt,e), balanced DVE~530us / ACT~570us, both < PE:
      zb = zp + bias_bcast      (DVE tensor_tensor, PSUM src, 1x, ~1.25us)
      per contribution (28 per bt over 12 experts):
        t/acc = Relu(sel_col * zb)   (ACT, per-partition scale AP; s>0 so
                                      s*relu(z)=relu(s*z); ~1.26us)
        acc += t                      (DVE fp16 tensor_tensor, 2x mode ~0.72us)
    First contribution per accumulator writes it directly; e=11 contributions
    DMA each finished [128,1024] fp16 acc straight to DRAM.
  - fp16 everywhere off-PSUM (measured rel err ~1.2e-3 vs fp32 reference).
"""

import numpy as np

import concourse.bacc as bacc
import concourse.mybir as mybir
import concourse.tile as tile
from concourse.bass_utils import run_bass_kernel_spmd

F32 = mybir.dt.float32
F16 = mybir.dt.float16
AL = mybir.AluOpType
AF = mybir.ActivationFunctionType

N_CORES = 8
B, D, O = 16384, 1024, 1024
BC = B // N_CORES
NE = 12  # experts
NG = 28  # gate columns: 8 (task0) + 8 (task1) + 12 (shared)
SEGS = ((0, 8), (8, 16), (16, 28))
G = 8  # batch tiles per accumulator group
KB = D // 128
N_BT = BC // 128
N_GRP = N_BT // G


def _contribs(e):
    """(out_k, sel_col) pairs for expert e. Gate col order: t0=[g0e0..3,g2e0..3],
    t1=[g1e0..3,g2e0..3], shared=[g0,g1,g2]."""
    if e < 4:
        return [(0, e), (2, 16 + e)]
    if e < 8:
        return [(1, 8 + (e - 4)), (2, 20 + (e - 4))]
    return [(0, 4 + (e - 8)), (1, 12 + (e - 8)), (2, 24 + (e - 8))]


def _build(reps=1):
    dt = F16

    nc = bacc.Bacc("TRN2", target_bir_lowering=False, debug=False)

    xT_d = nc.dram_tensor("xT", [D, BC], dt, kind="ExternalInput")
    w_d = nc.dram_tensor("We", [NE, D, O], dt, kind="ExternalInput")
    b_d = nc.dram_tensor("Be", [1, NE, O], dt, kind="ExternalInput")
    wg_d = nc.dram_tensor("Wgc", [D, NG], dt, kind="ExternalInput")
    bg_d = nc.dram_tensor("bgc", [1, NG], dt, kind="ExternalInput")
    out_d = [
        nc.dram_tensor(f"out{k}", [BC, O], F16, kind="ExternalOutput")
        for k in range(3)
    ]

    with tile.TileContext(nc) as tc:
        with (
            tc.tile_pool(name="big", bufs=1) as big,
            tc.tile_pool(name="wpool", bufs=2) as wpool,
            tc.tile_pool(name="accp", bufs=1) as accp,
            tc.tile_pool(name="tmpp", bufs=4) as tmpp,
            tc.tile_pool(name="gatep", bufs=2) as gatep,
            tc.tile_pool(name="psum", bufs=3, space="PSUM") as psum,
            tc.tile_pool(name="psumg", bufs=2, space="PSUM") as psumg,
        ):
            # --- resident staging (x split per k-slice so the first expert
            # chains can start as soon as slice 0 lands) ---
            x_sb = big.tile([128, KB, BC], dt)
            xT_r = xT_d.ap().rearrange("(k p) b -> p k b", p=128)
            for k in range(KB):
                nc.sync.dma_start(x_sb[:, k : k + 1, :], xT_r[:, k : k + 1, :])
            wg_sb = big.tile([128, KB, NG], dt)
            nc.sync.dma_start(
                wg_sb[:], wg_d.ap().rearrange("(k p) g -> p k g", p=128)
            )
            bg_sb = big.tile([1, NG], dt)
            nc.sync.dma_start(bg_sb[:], bg_d.ap())
            be_sb = big.tile([1, NE, O], dt)
            nc.sync.dma_start(be_sb[:], b_d.ap())
            ones_sb = big.tile([1, 128], dt)
            nc.vector.memset(ones_sb[:], 1.0)

            sel_sb = big.tile([128, N_BT, NG], F32)
            bb_sb = big.tile([128, NE, O], dt)

            for _rep in range(reps):
                _emit_body(
                    nc, x_sb, wg_sb, bg_sb, be_sb, ones_sb, sel_sb, bb_sb,
                    wpool, accp, tmpp, gatep, psum, psumg, w_d, out_d, dt,
                )

    nc.compile()
    return nc


def _emit_body(
    nc, x_sb, wg_sb, bg_sb, be_sb, ones_sb, sel_sb, bb_sb,
    wpool, accp, tmpp, gatep, psum, psumg, w_d, out_d, dt,
):
    # --- broadcast bias tiles: bb[e] = ones(128,1) x b[e] ---
    for e in range(NE):
        for h in range(2):
            pb = psumg.tile([128, 512], F32, tag="aux")
            nc.tensor.matmul(
                pb[:],
                ones_sb[:],
                be_sb[:, e, h * 512 : (h + 1) * 512],
                start=True,
                stop=True,
            )
            nc.scalar.activation(
                bb_sb[:, e, h * 512 : (h + 1) * 512], pb[:], AF.Copy
            )

    # --- experts + gated accumulation; gates ride inside the e=0 chains ---
    for grp in range(N_GRP):
        bt0 = grp * G
        accs = {}
        for e in range(NE):
            w_sb = wpool.tile([128, KB, O], dt, tag="w")
            nc.sync.dma_start(
                w_sb[:],
                w_d.ap()[e].rearrange("(k p) o -> p k o", p=128),
            )
            for bt in range(bt0, bt0 + G):
                zp = psum.tile([128, O], F32)
                pg = None
                if e == 0:
                    pg = psumg.tile([128, 512], F32, tag="aux")
                for k in range(KB):
                    xs = x_sb[:, k, bt * 128 : (bt + 1) * 128]
                    for h in range(2):
                        nc.tensor.matmul(
                            zp[:, h * 512 : (h + 1) * 512],
                            xs,
                            w_sb[:, k, h * 512 : (h + 1) * 512],
                            start=(k == 0),
                            stop=(k == KB - 1),
                        )
                    if e == 0:
                        nc.tensor.matmul(
                            pg[:, :NG],
                            xs,
                            wg_sb[:, k, :],
                            start=(k == 0),
                            stop=False,
                        )
                if e == 0:
                    # gate bias + segment softmax -> sel_sb[:, bt, :]
                    nc.tensor.matmul(
                        pg[:, :NG], ones_sb[:], bg_sb[:],
                        start=False, stop=True,
                    )
                    et = gatep.tile([128, NG], F32)
                    nc.scalar.activation(et[:], pg[:, :NG], AF.Exp)
                    for s0, s1 in SEGS:
                        den = gatep.tile([128, 1], F32, tag="den")
                        nc.vector.tensor_reduce(
                            den[:], et[:, s0:s1], mybir.AxisListType.X, AL.add
                        )
                        rden = gatep.tile([128, 1], F32, tag="rden")
                        nc.vector.reciprocal(rden[:], den[:])
                        nc.vector.tensor_scalar(
                            sel_sb[:, bt, s0:s1], et[:, s0:s1], rden[:],
                            None, AL.mult,
                        )
                zb = tmpp.tile([128, O], dt, tag="zb")
                nc.vector.tensor_tensor(zb[:], zp[:], bb_sb[:, e, :], AL.add)
                for k, col in _contribs(e):
                    sc = sel_sb[:, bt, col : col + 1]
                    key = (k, bt)
                    if key not in accs:
                        a = accp.tile([128, O], dt, tag=f"acc{k}_{bt - bt0}")
                        accs[key] = a
                        nc.scalar.activation(a[:], zb[:], AF.Relu, 0.0, sc)
                    else:
                        a = accs[key]
                        t = tmpp.tile([128, O], dt, tag="t")
                        nc.scalar.activation(t[:], zb[:], AF.Relu, 0.0, sc)
                        nc.vector.tensor_tensor(a[:], a[:], t[:], AL.add)
                    if e == NE - 1:
                        nc.sync.dma_start(
                            out_d[k].ap()[bt * 128 : (bt + 1) * 128, :],
                            a[:],
                        )


_NC_CACHE = None


def make_in_maps(x, W, b, Wg, bg, Wgs, bgs):
    x = np.asarray(x, dtype=np.float32)
    np_dt = np.float16
    shared = {
        "We": np.ascontiguousarray(np.asarray(W).reshape(NE, D, O)).astype(np_dt),
        "Be": np.asarray(b).reshape(1, NE, O).astype(np_dt),
        "Wgc": np.concatenate(
            [np.asarray(Wg)[0], np.asarray(Wg)[1], np.asarray(Wgs)], axis=1
        ).astype(np_dt),
        "bgc": np.concatenate(
            [np.asarray(bg)[0], np.asarray(bg)[1], np.asarray(bgs)]
        )[None, :].astype(np_dt),
    }
    in_maps = []
    for c in range(N_CORES):
        m = dict(shared)
        m["xT"] = np.ascontiguousarray(
            x[c * BC : (c + 1) * BC].T
        ).astype(np_dt)
        in_maps.append(m)
    return in_maps


def _gather(res):
    return tuple(
        np.concatenate(
            [res.results[c][f"out{k}"] for c in range(N_CORES)], axis=0
        ).astype(np.float32)
        for k in range(3)
    )


def kernel(x, W, b, Wg, bg, Wgs, bgs):
    global _NC_CACHE
    if _NC_CACHE is None:
        _NC_CACHE = _build()
    nc = _NC_CACHE

    in_maps = make_in_maps(x, W, b, Wg, bg, Wgs, bgs)
    res = run_bass_kernel_spmd(nc, in_maps, list(range(N_CORES)))
    return _gather(res)
